# revision 22
# baseline (speedup 1.0000x reference)
"""Causal self-attention (dense transformer block) on 8 Trainium2 NeuronCores.

Problem: B=2, T=2048, C=1024, H=16 heads, D=64.
    qkv = x @ w_attn + b_attn ; causal softmax attention ; out = y @ w_proj + b_proj

Sharding: 2-way data parallel on batch x 4-way tensor parallel on heads
(4 heads per core). c_attn is column-parallel, c_proj row-parallel; the
row-parallel all-reduce (sum of 4 partials per batch) + b_proj add happen on
the host at gather time.

v2 design (single fused pipeline, engine-balanced):
  - 5 rounds r=0..4. Round r computes QKV for token chunk ti=r (r<=3) and
    runs attention units (h, qc) for qc=r-1 interleaved, so ScalarE exp
    always overlaps PE work across the whole kernel.
  - x and QKV weights are bf16 (halves input DMA); Q/K kept f32(r) in SBUF,
    scores computed in f32r; P, V, y are bf16.
  - PV is computed transposed: ot[q,(d|l)] += ptT[k,q].T @ v[k,(d|l)] with a
    bf16 moving operand (65 cols @ 1 cycle/row) - half the PE columns of the
    [d,q]-major form. Per-head ones-column in V accumulates the softmax
    denominator l.
  - Normalization is per-PARTITION (1/l via DVE reciprocal + GPSIMD
    tensor_scalar), then a DMA-engine XBAR transpose (dma_start_transpose)
    produces the head-major y.T layout the projection needs: no PE or
    partition-broadcast cost.
  - Engine placement: PE matmuls only; Act exp only; DVE = QK bias-add,
    causal mask (bf16 4x mode), reciprocals, proj PSUM->SBUF staging;
    V copies + normalize also DVE (GPSIMD cannot touch PSUM). SP issues
    all DMAs; output stores are deferred to each round's end so their
    head-of-line waits never delay a transpose the projection needs.
  - PSUM: qk ring(2) + shared V/proj ring(2) + score ring(2) + ot(2) = 8;
    each PV qt-group is contiguous per PSUM bank (one open accum group/bank).
"""

from contextlib import ExitStack

import ml_dtypes
import numpy as np

import concourse.bacc as bacc
import concourse.bass as bass
import concourse.mybir as mybir
import concourse.tile as tile
from concourse import bass_utils

B, T, C, H = 2, 2048, 1024, 16
D = 64
NH = 4                      # heads per core
NP = NH // 2                # head pairs
N_CORES = 8
P = 128
CHX = C // P                # 8 x/w contraction chunks
NTC = T // 512              # 4 t-chunks
VW = NH * (D + 1)           # 260: V width incl. per-head ones column
F32 = mybir.dt.float32
F32R = mybir.dt.float32r
BF16 = mybir.dt.bfloat16

# round -> list of (head, q-chunk) attention units; unit (h, qc) is legal in
# any round >= qc (after that round's K chains). Spread so per-round Act(exp)
# stays below per-round PE work. Pairs (2j, 2j+1) stay together in order.
SCHED = {
    0: [(0, 0), (1, 0)],
    1: [(2, 0), (3, 0), (0, 1), (1, 1)],
    2: [(2, 1), (3, 1), (0, 2), (1, 2)],
    3: [(2, 2), (3, 2), (0, 3), (1, 3)],
    4: [(2, 3), (3, 3)],
}
# round -> slab projected via the round-distributed path; slab 3 is projected
# inline right behind its last unit's transposes (PROJ_INLINE).
PROJ_ROUND = {2: 0, 3: 1, 4: 2}
PROJ_INLINE = {(3, 3): 3}


def build_tile_kernel(tc, ins, out):
    nc = tc.nc
    scale = 1.0 / np.sqrt(D)

    xtb = ins["xtb"]    # [C, T] bf16 (x_b.T)
    wqa = ins["wqa"]    # [C, NH*D] bf16
    wka = ins["wka"]    # [C, NH*D] bf16
    wva = ins["wva"]    # [(CHX+1)*P, VW] bf16 (row C = bias / ones-col)
    wp = ins["wp"]      # [NH*D, C] bf16
    msk = ins["msk"]    # [P, 4*512] bf16 diagonal causal masks (row0 j0 = ones)
    eye = ins["eye"]    # [P, P] bf16 identity (PE transposes at the tail)
    bqk = ins["bqk"]    # [P, 4] f32 per-pair Q/K biases

    with ExitStack() as stk:
        pool = lambda name, bufs, space="SBUF": stk.enter_context(
            tc.tile_pool(name=name, bufs=bufs, space=space)
        )
        const = pool("const", 1)
        xpool = pool("xc", 2)
        qkv_sb = pool("qkv", 1)
        pt_pool = pool("pt", 40)
        stg_pool = pool("stg", 8)
        rc_pool = pool("rc", 8)
        ost_pool = pool("ost", 10)
        qk_ps = pool("qk_ps", 2, "PSUM")
        big_ps = pool("big_ps", 2, "PSUM")   # shared: V chains + projection
        st_ps = pool("st_ps", 2, "PSUM")
        ot_ps = pool("ot_ps", 2, "PSUM")

        # --- constant loads, all on the SP (sync) HWDGE queue, ordered by
        # first use: bqk+wqa before x chunk 0, wp (projection) last. ---
        wqa_sb = const.tile([P, CHX, NH * D], BF16, tag="wqa")
        wqa_r = wqa.rearrange("(c p) n -> p c n", p=P)
        nc.sync.dma_start(wqa_sb[:, 0:1, :], wqa_r[:, 0:1, :])
        nc.sync.dma_start(wqa_sb[:, 1:4, :], wqa_r[:, 1:4, :])

        # --- persistent activations ---
        qt_all = qkv_sb.tile([P, NP, T], F32R, tag="qt")  # [pair 2x64, pair, T]
        kt_all = qkv_sb.tile([P, NP, T], F32R, tag="kt")
        v_all = qkv_sb.tile([P, NTC * 4, VW], BF16, tag="v")   # [t, tt, d|l]
        yt_all = qkv_sb.tile([P, NP, T], BF16, tag="yt")  # [pair 2x64, pair, T]

        xr = xtb.rearrange("(c p) t -> p c t", p=P)
        xc_tiles = {}

        def issue_xc(r):
            xcr = xpool.tile([P, CHX, 512], BF16, tag="xc")
            nc.sync.dma_start(xcr[:, 0:4, :], xr[:, 0:4, bass.ts(r, 512)])
            nc.sync.dma_start(xcr[:, 4:8, :], xr[:, 4:8, bass.ts(r, 512)])
            xc_tiles[r] = xcr

        xc0 = xpool.tile([P, CHX, 512], BF16, tag="xc")
        nc.sync.dma_start(xc0[:, 0:2, :], xr[:, 0:2, 0:512])
        nc.sync.dma_start(wqa_sb[:, 4:8, :], wqa_r[:, 4:8, :])
        nc.sync.dma_start(xc0[:, 2:5, :], xr[:, 2:5, 0:512])
        nc.sync.dma_start(xc0[:, 5:8, :], xr[:, 5:8, 0:512])
        xc_tiles[0] = xc0
        bqk_sb = const.tile([P, 4], F32, tag="bqk")
        nc.sync.dma_start(bqk_sb[:], bqk[:, :])
        wka_sb = const.tile([P, CHX, NH * D], BF16, tag="wka")
        nc.sync.dma_start(wka_sb[:], wka.rearrange("(c p) n -> p c n", p=P))
        wva_sb = const.tile([P, CHX + 1, VW], BF16, tag="wva")
        nc.sync.dma_start(wva_sb[:], wva.rearrange("(c p) n -> p c n", p=P))
        msk_sb = const.tile([P, 4, 512], BF16, tag="msk")
        nc.sync.dma_start(msk_sb[:], msk.rearrange("p (j f) -> p j f", j=4))
        wp_sb = const.tile([P, NP, C], BF16, tag="wp")
        nc.sync.dma_start(wp_sb[:], wp.rearrange("(c p) n -> p c n", p=P))
        eye_sb = const.tile([P, P], BF16, tag="eye")
        nc.sync.dma_start(eye_sb[:], eye[:, :])

        def emit_qk(r, which, jj):
            """Q (which=0) or K (which=1) chain for pair jj, t-chunk r."""
            w_sb, dst = ((wqa_sb, qt_all), (wka_sb, kt_all))[which]
            xcr = xc_tiles[r]
            ps = qk_ps.tile([P, 512], F32, tag="qk")
            for c in range(CHX):
                nc.tensor.matmul(
                    ps[:], w_sb[:, c, bass.ts(jj, P)], xcr[:, c, :],
                    start=(c == 0), stop=(c == CHX - 1),
                )
            nc.vector.tensor_scalar_add(
                dst[:, jj, bass.ts(r, 512)], ps[:],
                bqk_sb[:, which * NP + jj : which * NP + jj + 1],
            )

        def emit_v(r, tt):
            """V rows for token tile r*4+tt, all heads + ones cols."""
            xcr = xc_tiles[r]
            ps = big_ps.tile([P, 512], F32, tag="big")
            pv = ps[:, 0:VW]
            for c in range(CHX):
                nc.tensor.matmul(
                    pv, xcr[:, c, bass.ts(tt, P)], wva_sb[:, c, :],
                    start=(c == 0), stop=False,
                )
            # rank-1 bias/ones row via 1-partition matmul (msk row0/j0 is ones)
            nc.tensor.matmul(
                pv, msk_sb[0:1, 0, 0:P], wva_sb[0:1, CHX, :],
                start=False, stop=True,
            )
            nc.vector.tensor_copy(v_all[:, r * 4 + tt, :], pv)
            if tt == 3:
                v_rounds[0] = r + 1

        cur_stg = {}
        s_alt = [0]  # alternate S tiles across st/qk rings (deeper exp buffer)
        live = []  # units with pending PV qt-groups (PSUM: one open group/bank)
        v_rounds = [0]  # rounds of V chunks fully emitted (pump gate)

        def finish_qt(u, qt):
            """Normalize y[:,qt] by 1/l into pair staging; transpose + (for the
            final slab) projection as soon as the pair's qt columns are done."""
            h, qc, ot = u["h"], u["qc"], u["ot"]
            hb, hj = (h % 2) * D, h // 2
            rc = rc_pool.tile([P, 1], F32, tag="rc")
            nc.vector.reciprocal(rc[:], ot[:, qt, D : D + 1])
            if h % 2 == 0:
                cur_stg[(hj, qt)] = stg_pool.tile(
                    [P, P], BF16, tag="stg", name="stg"
                )
            stg = cur_stg[(hj, qt)]
            nc.vector.tensor_scalar_mul(
                stg[:, hb : hb + D], ot[:, qt, 0:D], rc[:]
            )
            if h % 2 == 1:
                if u.get("pe_tr"):
                    # tail: PE transpose (stg.T @ I) avoids ~2.5us XBAR latency
                    tp = st_ps.tile([P, 512], F32, tag="st")
                    nc.tensor.matmul(
                        tp[:, 0:P], stg[:], eye_sb[:], start=True, stop=True
                    )
                    nc.vector.tensor_copy(
                        yt_all[:, hj, bass.ts(qc * 4 + qt, P)], tp[:, 0:P]
                    )
                else:
                    nc.sync.dma_start_transpose(
                        yt_all[:, hj, bass.ts(qc * 4 + qt, P)], stg[:]
                    )
                if u.get("proj_after") is not None:
                    emit_proj(u["proj_after"], 2 * qt, inline_out=True)
                    emit_proj(u["proj_after"], 2 * qt + 1, inline_out=True)

        def pump_pv(force=False):
            """Emit the oldest unit's next PV qt-group (contiguous in ot bank)."""
            if not live or not live[0]["pvgs"]:
                return
            if live[0]["qc"] >= v_rounds[0] and not force:
                return  # this unit's V chunk isn't emitted yet
            u = live[0]
            qt = u["pvgs"].pop(0)
            h, qc = u["h"], u["qc"]
            last = qc * 4 + qt
            for kt in range(last + 1):
                nc.tensor.matmul(
                    u["ot"][:, qt, :],
                    u["pts"][kt][:, bass.ts(qt, P)],
                    v_all[:, kt, h * (D + 1) : (h + 1) * (D + 1)],
                    start=(kt == 0), stop=(kt == last),
                )
            finish_qt(u, qt)
            if not u["pvgs"]:
                live.pop(0)

        def emit_unit(h, qc, after_step=None):
            """S/exp/mask phase for unit (h, qc); PV of older units pumped in."""
            hb, hj = (h % 2) * D, h // 2
            nkt = 4 * (qc + 1)
            pts = []
            for kt in range(nkt):
                j = kt - (nkt - 4)
                lo = max(j, 0) * P
                s_alt[0] ^= 1
                spool = st_ps if s_alt[0] else qk_ps
                st = spool.tile([P, 512], F32, tag="st" if s_alt[0] else "qk",
                                name="st")
                nc.tensor.matmul(
                    st[:, lo:512],
                    kt_all[hb : hb + D, hj, bass.ts(kt, P)],
                    qt_all[hb : hb + D, hj, qc * 512 + lo : (qc + 1) * 512],
                    start=True, stop=True,
                )
                pt = pt_pool.tile([P, 512], BF16, tag="pt")
                nc.scalar.activation(
                    pt[:, lo:512], st[:, lo:512],
                    mybir.ActivationFunctionType.Exp, scale=float(scale),
                )
                if j >= 0:
                    nc.vector.tensor_mul(
                        pt[:, lo:512], pt[:, lo:512], msk_sb[:, j, lo:512]
                    )
                pts.append(pt)
                pump_pv()
                if after_step is not None:
                    after_step()
            ot = ot_ps.tile([P, 4, D + 1], F32, tag="ot")
            live.append({
                "h": h, "qc": qc, "ot": ot, "pts": pts, "pvgs": [0, 1, 2, 3],
                "proj_after": PROJ_INLINE.get((h, qc)),
                "pe_tr": (h, qc) == (3, 3),
            })

        pending_outs = []

        def flush_outs():
            while pending_outs:
                tt, cc, st = pending_outs.pop(0)
                nc.sync.dma_start(out[bass.ts(tt, P), bass.ts(cc, 512)], st[:])

        def emit_proj(pq, g, inline_out=False):
            """Projection group g (tt=pq*4+g//2, cc=g%2) for slab pq."""
            tt, cc = pq * 4 + g // 2, g % 2
            ps = big_ps.tile([P, 512], F32, tag="big")
            for jj in range(NP):
                nc.tensor.matmul(
                    ps[:], yt_all[:, jj, bass.ts(tt, P)],
                    wp_sb[:, jj, bass.ts(cc, 512)],
                    start=(jj == 0), stop=(jj == NP - 1),
                )
            st = ost_pool.tile([P, 512], F32, tag="ost")
            nc.vector.tensor_copy(st[:], ps[:])
            if inline_out:
                nc.sync.dma_start(out[bass.ts(tt, P), bass.ts(cc, 512)], st[:])
            else:
                pending_outs.append((tt, cc, st))

        # ---------------- the fused pipeline ----------------
        # Per round, PE work is ordered to feed the Act engine (exp) first:
        # S-phases of units whose inputs exist, then this round's QKV chains,
        # then the units needing them; V chains and projection (pure-PE,
        # deferrable) fill the round's tail while Act catches up.
        for r in range(5):
            units = SCHED[r]
            has_qkv = r <= 3
            if r == 0:
                emit_qk(0, 0, 0)
                emit_qk(0, 0, 1)
                emit_qk(0, 1, 0)
                emit_qk(0, 1, 1)
                issue_xc(1)
                for tt in range(4):
                    emit_v(0, tt)
                for h, qc in units:
                    emit_unit(h, qc)
                continue
            for h, qc in units:
                if qc < r:
                    emit_unit(h, qc)
            if has_qkv:
                emit_qk(r, 0, 0)
                emit_qk(r, 0, 1)
                emit_qk(r, 1, 0)
                emit_qk(r, 1, 1)
                if r + 1 <= 3:
                    issue_xc(r + 1)
            for h, qc in units:
                if qc == r:
                    emit_unit(h, qc)
            if has_qkv:
                for tt in range(4):
                    emit_v(r, tt)
            pq = PROJ_ROUND.get(r, -1)
            if 0 <= pq < 3:
                for g in range(8):
                    emit_proj(pq, g)
                    if r == 4:
                        pump_pv(force=True)
            flush_outs()
        while live:
            pump_pv(force=True)
        flush_outs()


def make_shard_inputs(x_b, w_attn, b_attn, w_proj, h0):
    """Per-core input dict for batch slice x_b [T, C] and heads h0..h0+NH-1."""
    bf = ml_dtypes.bfloat16
    xtb = np.ascontiguousarray(x_b.T).astype(bf)

    qs = slice(h0 * D, (h0 + NH) * D)
    ks = slice(C + h0 * D, C + (h0 + NH) * D)
    wqa = np.ascontiguousarray(w_attn[:, qs]).astype(bf)
    wka = np.ascontiguousarray(w_attn[:, ks]).astype(bf)

    wva = np.zeros((C + P, VW), dtype=np.float32)
    for h in range(NH):
        vs = slice(2 * C + (h0 + h) * D, 2 * C + (h0 + h + 1) * D)
        wva[:C, h * (D + 1) : h * (D + 1) + D] = w_attn[:, vs]
        wva[C, h * (D + 1) : h * (D + 1) + D] = b_attn[vs]
        wva[C, h * (D + 1) + D] = 1.0  # ones column -> softmax denominator
    wva = wva.astype(bf)

    wp = np.ascontiguousarray(w_proj[h0 * D : (h0 + NH) * D, :]).astype(bf)

    msk = np.zeros((P, 4 * 512), dtype=np.float32)
    p = np.arange(P)[:, None]
    f = np.arange(512)[None, :]
    for j in range(4):
        msk[:, j * 512 : (j + 1) * 512] = (j * P + p <= f).astype(np.float32)
    msk = msk.astype(bf)

    bqk = np.zeros((P, 4), dtype=np.float32)
    for j in range(NP):
        bqk[:, j] = b_attn[(h0 + 2 * j) * D : (h0 + 2 * j + 2) * D]
        bqk[:, NP + j] = b_attn[C + (h0 + 2 * j) * D : C + (h0 + 2 * j + 2) * D]

    return {
        "xtb": xtb, "wqa": wqa, "wka": wka, "wva": wva,
        "wp": wp, "msk": msk, "bqk": bqk,
        "eye": np.eye(P, dtype=np.float32).astype(bf),
    }


_NC_CACHE = {}


def _build_nc():
    if "nc" in _NC_CACHE:
        return _NC_CACHE["nc"]
    nc = bacc.Bacc("TRN2", target_bir_lowering=False, debug=False)
    in_specs = {
        "xtb": ((C, T), BF16),
        "wqa": ((C, NH * D), BF16),
        "wka": ((C, NH * D), BF16),
        "wva": ((C + P, VW), BF16),
        "wp": ((NH * D, C), BF16),
        "msk": ((P, 4 * 512), BF16),
        "bqk": ((P, 4), F32),
        "eye": ((P, P), BF16),
    }
    in_aps = {
        k: nc.dram_tensor(k, list(s), dt, kind="ExternalInput").ap()
        for k, (s, dt) in in_specs.items()
    }
    out_ap = nc.dram_tensor("out", [T, C], F32, kind="ExternalOutput").ap()
    with tile.TileContext(nc) as tc:
        build_tile_kernel(tc, in_aps, out_ap)
    nc.compile()
    _NC_CACHE["nc"] = nc
    return nc


def _run(inputs, trace=False):
    x = np.ascontiguousarray(inputs["x"], dtype=np.float32)
    w_attn = np.ascontiguousarray(inputs["w_attn"], dtype=np.float32)
    b_attn = np.ascontiguousarray(inputs["b_attn"], dtype=np.float32)
    w_proj = np.ascontiguousarray(inputs["w_proj"], dtype=np.float32)
    b_proj = np.ascontiguousarray(inputs["b_proj"], dtype=np.float32)

    nc = _build_nc()
    in_maps = [
        make_shard_inputs(x[c // 4], w_attn, b_attn, w_proj, (c % 4) * NH)
        for c in range(N_CORES)
    ]
    res = bass_utils.run_bass_kernel_spmd(
        nc, in_maps, core_ids=list(range(N_CORES)), trace=trace
    )
    out = np.zeros((B, T, C), dtype=np.float64)
    for c in range(N_CORES):
        out[c // 4] += res.results[c]["out"].astype(np.float64)
    out += b_proj.astype(np.float64)
    return out.astype(np.float32), res


def kernel(**inputs):
    out, _ = _run(inputs)
    return out
